# revision 6
# baseline (speedup 1.0000x reference)
"""Trainium2 Bass kernel for nn_Model3_4355096838495 (topk_masking).

Computes: relu-linear transforms of comments and srt table, dense attention
scores, top-2 srt rows per comment (softmax is monotonic, so top-k on raw
scores equals top-k on softmax weights), gather of the selected transformed
srt rows.

Distribution across 8 NeuronCores:
- srt table rows sharded (1024/core): each core computes an exact transposed
  transform ts.T of its shard via 3-term fp16-pair matmuls (fp32-grade).
- comments sharded (512/core) for the comment transform; the fp16-pair tc.T
  shards are AllGathered (16.8MB) while the srt transform runs.
- scores [4096 x own-1024] per core via 3-term fp16-pair matmuls; per-row
  top-8 candidates via DVE max/max_index on the exact fp32 scores.
- tiny candidate AllGather, exact merge of 8x8 candidates for own comments,
- final values recomputed exactly in fp32 from gathered hidden_states_srt
  rows (indirect DMA + PE transpose + fp32 matmul + bias + relu).

Precision strategy (validated on HW): PE fp32r/bf16/fp16 all round operands
to ~11-12 mantissa bits; a 3-term fp16-pair product (x1y1+x1y2+x2y1) is
fp32-grade (~5e-7) and runs at 3 cyc/row vs fp32's 4. Top-2 selection must
match the fp32 reference exactly (score gaps down to ~2e-6 relative), which
the 3-term scheme achieves with zero flipped rows on the reference data.
"""
import os
import sys
from dataclasses import dataclass

import numpy as np

try:
    import concourse.bass as bass  # noqa: F401
except ImportError:  # pragma: no cover
    sys.path.insert(0, "/opt/trn_rl_repo")

import concourse.bass as bass
import concourse.mybir as mybir
from concourse import bacc
from concourse.tile import TileContext
from concourse.bass_utils import run_bass_kernel_spmd
from concourse.masks import make_identity

F32 = mybir.dt.float32
F16 = mybir.dt.float16
U32 = mybir.dt.uint32
P = 128
# Sentinel for masked-index min-reduction. Must be a power of two larger than
# any srt index so that (idx - BIG) + BIG is exact in fp32, and large enough
# that subtracting it from a score pushes the value below every real score.
BIG = 16384.0


@dataclass(frozen=True)
class Cfg:
    b: int = 4096        # comments
    n_srt: int = 8192    # srt rows
    dc: int = 768        # comment hidden
    ds: int = 1024       # srt hidden
    n_cores: int = 8

    @property
    def bs(self):  # comment shard per core
        return self.b // self.n_cores

    @property
    def ns(self):  # srt shard per core
        return self.n_srt // self.n_cores


FULL = Cfg()


def _build(cfg: Cfg):
    """Build the SPMD Bass program (one program, per-core data differs)."""
    nc_cores = cfg.n_cores
    KT_C = cfg.dc // P          # k-tiles for comment transform
    KT_S = cfg.ds // P          # k-tiles over hidden dim ds
    OT = cfg.ds // P            # output o-tiles of transforms
    BS, NS = cfg.bs, cfg.ns
    B_TILES = cfg.b // P        # score b-tiles (all comments)
    OWN_BT = BS // P            # own comment b-tiles
    N_CH = max(NS // 512, 1)    # 512-wide free chunks of own srt shard
    NCW = min(512, NS)          # chunk width
    BCW = min(512, BS)          # comment-transform free width
    B_CH = max(BS // 512, 1)
    OCW = min(512, cfg.ds)
    O_CH = max(cfg.ds // 512, 1)

    nc = bacc.Bacc("TRN2", target_bir_lowering=False, debug=False,
                   num_devices=nc_cores)

    def din(name, shape, dt):
        return nc.dram_tensor(name, shape, dt, kind="ExternalInput").ap()

    wc1 = din("wc1", [cfg.dc, cfg.ds], F16)
    wc2 = din("wc2", [cfg.dc, cfg.ds], F16)
    hc1 = din("hc1", [cfg.dc, BS], F16)
    hc2 = din("hc2", [cfg.dc, BS], F16)
    ws1 = din("ws1", [cfg.ds, cfg.ds], F16)
    ws2 = din("ws2", [cfg.ds, cfg.ds], F16)
    hs1 = din("hs1", [cfg.ds, NS], F16)
    hs2 = din("hs2", [cfg.ds, NS], F16)
    ws_f32 = din("ws_f32", [cfg.ds, cfg.ds], F32)
    hs_full = din("hs_full", [cfg.n_srt, cfg.ds], F32)
    bc_in = din("bc", [cfg.ds], F32)
    bs_in = din("bs", [cfg.ds], F32)
    core_off = din("core_off", [P, 1], F32)
    rm = din("rm", [P, OWN_BT * nc_cores], U32)

    sel_out = nc.dram_tensor("sel", [BS, 2, cfg.ds], F32,
                             kind="ExternalOutput").ap()

    with TileContext(nc) as tc:
        _body(tc, nc, cfg, locals())
    nc.compile()
    return nc


def _body(tc, nc, cfg: Cfg, v):
    nc_cores = cfg.n_cores
    KT_C = cfg.dc // P
    KT_S = cfg.ds // P
    OT = cfg.ds // P
    BS, NS = cfg.bs, cfg.ns
    B_TILES = cfg.b // P
    OWN_BT = BS // P
    N_CH = max(NS // 512, 1)
    NCW = min(512, NS)
    BCW = min(512, BS)
    B_CH = max(BS // 512, 1)
    OCW = min(512, cfg.ds)
    O_CH = max(cfg.ds // 512, 1)
    groups = [list(range(nc_cores))]

    from contextlib import ExitStack
    ctx = ExitStack()
    with ctx:
        const = ctx.enter_context(tc.tile_pool(name="const", bufs=1))
        dram = ctx.enter_context(tc.tile_pool(name="dram", bufs=1, space="DRAM"))

        # ---- constants ----
        ident = const.tile([P, P], F32)
        make_identity(nc, ident)
        ones_row = const.tile([1, P], F32)
        nc.vector.memset(ones_row[:], 1.0)
        b_com_sb = const.tile([P, OT], F32)
        nc.sync.dma_start(out=b_com_sb[:],
                          in_=v["bc_in"].rearrange("(t p) -> p t", p=P))
        b_srt_sb = const.tile([P, OT], F32)
        nc.sync.dma_start(out=b_srt_sb[:],
                          in_=v["bs_in"].rearrange("(t p) -> p t", p=P))
        b_srt_row = const.tile([1, cfg.ds], F32)
        nc.sync.dma_start(out=b_srt_row[:],
                          in_=v["bs_in"].rearrange("(o d) -> o d", o=1))
        core_off_sb = const.tile([P, 1], F32)
        nc.sync.dma_start(out=core_off_sb[:], in_=v["core_off"][:])
        rm_sb = const.tile([P, OWN_BT * nc_cores], U32)
        nc.sync.dma_start(out=rm_sb[:], in_=v["rm"][:])

        # DRAM scratch
        tc_b_in = dram.tile([2, cfg.ds, BS], F16)
        tc_ag = dram.tile([nc_cores, 2, cfg.ds, BS], F16, addr_space="Shared")
        cand_in = dram.tile([cfg.b, 16], F32)
        cand_ag = dram.tile([nc_cores, cfg.b, 16], F32, addr_space="Shared")

        # =========== Phase 1: comment transform (own shard) ===========
        with tc.tile_pool(name="tcph", bufs=1) as tp, \
             tc.tile_pool(name="tcps", bufs=4, space="PSUM") as pp, \
             tc.tile_pool(name="tcout", bufs=2) as op:
            wc1_sb = tp.tile([P, KT_C * cfg.ds], F16, tag="wc1")
            wc2_sb = tp.tile([P, KT_C * cfg.ds], F16, tag="wc2")
            hc1_sb = tp.tile([P, KT_C * BS], F16, tag="hc1")
            hc2_sb = tp.tile([P, KT_C * BS], F16, tag="hc2")
            for t_sb, t_dr, w in ((wc1_sb, v["wc1"], cfg.ds), (wc2_sb, v["wc2"], cfg.ds),
                                  (hc1_sb, v["hc1"], BS), (hc2_sb, v["hc2"], BS)):
                nc.sync.dma_start(
                    out=t_sb[:].rearrange("p (k w) -> p k w", k=KT_C),
                    in_=t_dr.rearrange("(k p) w -> p k w", p=P))

            for ot in range(OT):
                for bc in range(B_CH):
                    ps = pp.tile([P, BCW], F32, tag="ps")
                    terms = ((wc1_sb, hc1_sb), (wc1_sb, hc2_sb), (wc2_sb, hc1_sb))
                    n_mm = len(terms) * KT_C
                    i = 0
                    for lt, rt in terms:
                        for k in range(KT_C):
                            nc.tensor.matmul(
                                ps[:],
                                lhsT=lt[:, k * cfg.ds + ot * P:k * cfg.ds + (ot + 1) * P],
                                rhs=rt[:, k * BS + bc * BCW:k * BS + bc * BCW + BCW],
                                start=(i == 0), stop=(i == n_mm - 1))
                            i += 1
                    # exact fp32 relu(x+b) then split to fp16 pair
                    tcx = op.tile([P, BCW], F32, tag="tcx")
                    nc.scalar.activation(tcx[:], ps[:],
                                         mybir.ActivationFunctionType.Relu,
                                         bias=b_com_sb[:, ot:ot + 1])
                    h1 = op.tile([P, BCW], F16, tag="h1")
                    nc.vector.tensor_copy(h1[:], tcx[:])
                    u1 = op.tile([P, BCW], F32, tag="u1")
                    nc.scalar.copy(u1[:], h1[:])
                    h2 = op.tile([P, BCW], F16, tag="h2")
                    nc.vector.tensor_tensor(out=h2[:], in0=tcx[:], in1=u1[:],
                                            op=mybir.AluOpType.subtract)
                    nc.sync.dma_start(
                        out=tc_b_in[0, ot * P:(ot + 1) * P, bc * BCW:bc * BCW + BCW],
                        in_=h1[:])
                    nc.sync.dma_start(
                        out=tc_b_in[1, ot * P:(ot + 1) * P, bc * BCW:bc * BCW + BCW],
                        in_=h2[:])

        nc.gpsimd.collective_compute(
            "AllGather", mybir.AluOpType.bypass, replica_groups=groups,
            ins=[tc_b_in.opt()], outs=[tc_ag.opt()])

        # =========== Phase 2: srt transform (own shard) -> tsT pair in SBUF ==
        ts1_sb = const.tile([P, KT_S * NS], F16, tag="ts1")
        ts2_sb = const.tile([P, KT_S * NS], F16, tag="ts2")
        with tc.tile_pool(name="tsph", bufs=1) as tp, \
             tc.tile_pool(name="tsps", bufs=4, space="PSUM") as pp, \
             tc.tile_pool(name="tsout", bufs=2) as op:
            ws1_sb = tp.tile([P, KT_S * cfg.ds], F16, tag="ws1")
            ws2_sb = tp.tile([P, KT_S * cfg.ds], F16, tag="ws2")
            hs1_sb = tp.tile([P, KT_S * NS], F16, tag="hs1")
            hs2_sb = tp.tile([P, KT_S * NS], F16, tag="hs2")
            for t_sb, t_dr, w in ((ws1_sb, v["ws1"], cfg.ds), (ws2_sb, v["ws2"], cfg.ds),
                                  (hs1_sb, v["hs1"], NS), (hs2_sb, v["hs2"], NS)):
                nc.sync.dma_start(
                    out=t_sb[:].rearrange("p (k w) -> p k w", k=KT_S),
                    in_=t_dr.rearrange("(k p) w -> p k w", p=P))

            for ot in range(OT):
                for nch in range(N_CH):
                    ps = pp.tile([P, NCW], F32, tag="ps")
                    terms = ((ws1_sb, hs1_sb), (ws1_sb, hs2_sb), (ws2_sb, hs1_sb))
                    n_mm = len(terms) * KT_S
                    i = 0
                    for lt, rt in terms:
                        for k in range(KT_S):
                            nc.tensor.matmul(
                                ps[:],
                                lhsT=lt[:, k * cfg.ds + ot * P:k * cfg.ds + (ot + 1) * P],
                                rhs=rt[:, k * NS + nch * NCW:k * NS + nch * NCW + NCW],
                                start=(i == 0), stop=(i == n_mm - 1))
                            i += 1
                    tsx = op.tile([P, NCW], F32, tag="tsx")
                    nc.scalar.activation(tsx[:], ps[:],
                                         mybir.ActivationFunctionType.Relu,
                                         bias=b_srt_sb[:, ot:ot + 1])
                    dst1 = ts1_sb[:, ot * NS + nch * NCW:ot * NS + nch * NCW + NCW]
                    nc.vector.tensor_copy(dst1, tsx[:])
                    u1 = op.tile([P, NCW], F32, tag="u1")
                    nc.scalar.copy(u1[:], dst1)
                    dst2 = ts2_sb[:, ot * NS + nch * NCW:ot * NS + nch * NCW + NCW]
                    nc.vector.tensor_tensor(out=dst2, in0=tsx[:], in1=u1[:],
                                            op=mybir.AluOpType.subtract)

        # =========== Phase 3: scores + per-row top-8 ===========
        with tc.tile_pool(name="sclhs", bufs=4) as lp, \
             tc.tile_pool(name="scps", bufs=4, space="PSUM") as pp, \
             tc.tile_pool(name="scsb", bufs=3) as sp, \
             tc.tile_pool(name="sccand", bufs=3) as cp:
            for bt in range(B_TILES):
                rank = bt // OWN_BT
                lb = bt % OWN_BT
                lhs1 = lp.tile([P, KT_S * P], F16, tag="lhs1")
                lhs2 = lp.tile([P, KT_S * P], F16, tag="lhs2")
                nc.sync.dma_start(
                    out=lhs1[:].rearrange("p (k w) -> p k w", k=KT_S),
                    in_=tc_ag[rank, 0, :, lb * P:(lb + 1) * P]
                        .rearrange("(k p) w -> p k w", p=P))
                nc.sync.dma_start(
                    out=lhs2[:].rearrange("p (k w) -> p k w", k=KT_S),
                    in_=tc_ag[rank, 1, :, lb * P:(lb + 1) * P]
                        .rearrange("(k p) w -> p k w", p=P))
                scores = sp.tile([P, NS], F32, tag="scores")
                for nch in range(N_CH):
                    ps = pp.tile([P, NCW], F32, tag="ps")
                    terms = ((lhs1, ts1_sb), (lhs1, ts2_sb), (lhs2, ts1_sb))
                    n_mm = len(terms) * KT_S
                    i = 0
                    for lt, rt in terms:
                        for k in range(KT_S):
                            nc.tensor.matmul(
                                ps[:],
                                lhsT=lt[:, k * P:(k + 1) * P],
                                rhs=rt[:, k * NS + nch * NCW:k * NS + nch * NCW + NCW],
                                start=(i == 0), stop=(i == n_mm - 1))
                            i += 1
                    nc.scalar.copy(scores[:, nch * NCW:nch * NCW + NCW], ps[:])
                mx = cp.tile([P, 8], F32, tag="mx")
                nc.vector.max(out=mx[:], in_=scores[:])
                mi = cp.tile([P, 8], U32, tag="mi")
                nc.vector.max_index(out=mi[:], in_max=mx[:], in_values=scores[:])
                cand = cp.tile([P, 16], F32, tag="cand")
                nc.vector.tensor_copy(cand[:, 0:8], mx[:])
                nc.vector.tensor_copy(cand[:, 8:16], mi[:])  # u32 -> f32 cast
                nc.vector.tensor_scalar(
                    cand[:, 8:16], cand[:, 8:16], core_off_sb[:, 0:1], None,
                    op0=mybir.AluOpType.add)
                nc.sync.dma_start(out=cand_in[bt * P:(bt + 1) * P, :], in_=cand[:])

        nc.gpsimd.collective_compute(
            "AllGather", mybir.AluOpType.bypass, replica_groups=groups,
            ins=[cand_in.opt()], outs=[cand_ag.opt()])
        cand_flat = cand_ag.rearrange("r b s -> (r b) s")

        # =========== Phase 4: merge own comments' candidates ===========
        with tc.tile_pool(name="mg", bufs=2) as mp, \
             tc.tile_pool(name="mgsel", bufs=1) as selp:
            sel_u32 = selp.tile([P, OWN_BT * 2], U32, tag="sel")
            for obt in range(OWN_BT):
                ct = mp.tile([P, nc_cores * 16], F32, tag="ct")
                for q in range(nc_cores):
                    nc.gpsimd.indirect_dma_start(
                        out=ct[:, q * 16:(q + 1) * 16], out_offset=None,
                        in_=cand_flat,
                        in_offset=bass.IndirectOffsetOnAxis(
                            ap=rm_sb[:, obt * nc_cores + q:obt * nc_cores + q + 1],
                            axis=0))
                vals = ct[:].rearrange("p (q s) -> p q s", q=nc_cores)[:, :, 0:8]
                idxs = ct[:].rearrange("p (q s) -> p q s", q=nc_cores)[:, :, 8:16]
                nq = nc_cores * 8

                def bcast(x):
                    return x.rearrange("p (a o) -> p a o", o=1) \
                            .to_broadcast([P, nc_cores, 8])

                m1 = mp.tile([P, 1], F32, tag="m1")
                nc.vector.tensor_reduce(out=m1[:], in_=vals,
                                        axis=mybir.AxisListType.XY,
                                        op=mybir.AluOpType.max)
                eq = mp.tile([P, nq], F32, tag="eq")
                eqv = eq[:].rearrange("p (q s) -> p q s", q=nc_cores)
                nc.vector.tensor_tensor(out=eqv, in0=vals, in1=bcast(m1[:]),
                                        op=mybir.AluOpType.is_equal)
                # masked idx: eq ? idx : BIG  == eq*(idx-BIG) + BIG
                t1 = mp.tile([P, nq], F32, tag="t1")
                t1v = t1[:].rearrange("p (q s) -> p q s", q=nc_cores)
                nc.vector.tensor_scalar(t1v, idxs, -BIG, None,
                                        op0=mybir.AluOpType.add)
                nc.vector.tensor_tensor(out=t1v, in0=t1v, in1=eqv,
                                        op=mybir.AluOpType.mult)
                nc.vector.tensor_scalar(t1v, t1v, BIG, None,
                                        op0=mybir.AluOpType.add)
                i1 = mp.tile([P, 1], F32, tag="i1")
                nc.vector.tensor_reduce(out=i1[:], in_=t1v,
                                        axis=mybir.AxisListType.XY,
                                        op=mybir.AluOpType.min)
                # kill the winner (global idx unique): vals2 = vals - (idx==i1)*BIG
                k1 = mp.tile([P, nq], F32, tag="k1")
                k1v = k1[:].rearrange("p (q s) -> p q s", q=nc_cores)
                nc.vector.tensor_tensor(out=k1v, in0=idxs, in1=bcast(i1[:]),
                                        op=mybir.AluOpType.is_equal)
                nc.vector.tensor_scalar(k1v, k1v, BIG, None,
                                        op0=mybir.AluOpType.mult)
                v2 = mp.tile([P, nq], F32, tag="v2")
                v2v = v2[:].rearrange("p (q s) -> p q s", q=nc_cores)
                nc.vector.tensor_tensor(out=v2v, in0=vals, in1=k1v,
                                        op=mybir.AluOpType.subtract)
                m2 = mp.tile([P, 1], F32, tag="m2")
                nc.vector.tensor_reduce(out=m2[:], in_=v2v,
                                        axis=mybir.AxisListType.XY,
                                        op=mybir.AluOpType.max)
                nc.vector.tensor_tensor(out=eqv, in0=v2v, in1=bcast(m2[:]),
                                        op=mybir.AluOpType.is_equal)
                nc.vector.tensor_scalar(t1v, idxs, -BIG, None,
                                        op0=mybir.AluOpType.add)
                nc.vector.tensor_tensor(out=t1v, in0=t1v, in1=eqv,
                                        op=mybir.AluOpType.mult)
                nc.vector.tensor_scalar(t1v, t1v, BIG, None,
                                        op0=mybir.AluOpType.add)
                i2 = mp.tile([P, 1], F32, tag="i2")
                nc.vector.tensor_reduce(out=i2[:], in_=t1v,
                                        axis=mybir.AxisListType.XY,
                                        op=mybir.AluOpType.min)
                nc.vector.tensor_copy(sel_u32[:, obt * 2:obt * 2 + 1], i1[:])
                nc.vector.tensor_copy(sel_u32[:, obt * 2 + 1:obt * 2 + 2], i2[:])

            # =========== Phase 5: tail — recompute selected rows exactly ====
            R = OWN_BT * 2 * P  # selected rows
            with tc.tile_pool(name="tlg", bufs=2) as gp, \
                 tc.tile_pool(name="tlgt", bufs=1) as gtp, \
                 tc.tile_pool(name="tlw", bufs=1) as wp, \
                 tc.tile_pool(name="tlps", bufs=4, space="PSUM") as pp, \
                 tc.tile_pool(name="tlps2", bufs=4, space="PSUM") as pp2, \
                 tc.tile_pool(name="tlout", bufs=3) as op:
                ws32_sb = wp.tile([P, KT_S * cfg.ds], F32, tag="ws32")
                nc.sync.dma_start(
                    out=ws32_sb[:].rearrange("p (k w) -> p k w", k=KT_S),
                    in_=v["ws_f32"].rearrange("(k p) w -> p k w", p=P))
                hsgT = gtp.tile([P, KT_S * R], F32, tag="hsgT")
                for obt in range(OWN_BT):
                    for j in range(2):
                        rt = obt * 2 + j
                        g = gp.tile([P, cfg.ds], F32, tag="g")
                        nc.gpsimd.indirect_dma_start(
                            out=g[:], out_offset=None,
                            in_=v["hs_full"][:],
                            in_offset=bass.IndirectOffsetOnAxis(
                                ap=sel_u32[:, obt * 2 + j:obt * 2 + j + 1],
                                axis=0))
                        for dj in range(KT_S):
                            tp_ps = pp2.tile([P, P], F32, tag="tp")
                            nc.tensor.transpose(out=tp_ps[:],
                                                in_=g[:, dj * P:(dj + 1) * P],
                                                identity=ident[:])
                            nc.vector.tensor_copy(
                                hsgT[:, dj * R + rt * P:dj * R + (rt + 1) * P],
                                tp_ps[:])
                for rt in range(OWN_BT * 2):
                    obt, j = rt // 2, rt % 2
                    for oc in range(O_CH):
                        ps = pp.tile([P, OCW], F32, tag="mmps")
                        nc.tensor.matmul(
                            ps[:], lhsT=ones_row[0:1, :],
                            rhs=b_srt_row[0:1, oc * OCW:oc * OCW + OCW],
                            start=True, stop=False)
                        for k in range(KT_S):
                            nc.tensor.matmul(
                                ps[:],
                                lhsT=hsgT[:, k * R + rt * P:k * R + (rt + 1) * P],
                                rhs=ws32_sb[:, k * cfg.ds + oc * OCW:k * cfg.ds + oc * OCW + OCW],
                                start=False, stop=(k == KT_S - 1))
                        o_sb = op.tile([P, OCW], F32, tag="osb")
                        nc.scalar.activation(o_sb[:], ps[:],
                                             mybir.ActivationFunctionType.Relu,
                                             bias=0.0)
                        nc.sync.dma_start(
                            out=v["sel_out"][obt * P:(obt + 1) * P, j,
                                             oc * OCW:oc * OCW + OCW],
                            in_=o_sb[:])


# ---------------------------------------------------------------------------
# host side
# ---------------------------------------------------------------------------

def _f16_pair(x):
    x = np.ascontiguousarray(x, np.float32)
    h1 = x.astype(np.float16)
    h2 = (x - h1.astype(np.float32)).astype(np.float16)
    return h1, h2


def _host_prep(inputs, cfg: Cfg):
    hs = np.ascontiguousarray(np.asarray(inputs["hidden_states_srt"], np.float32))
    hc = np.ascontiguousarray(np.asarray(inputs["hidden_states_comments"], np.float32))
    Wc = np.ascontiguousarray(np.asarray(inputs["W_comment"], np.float32))
    bc = np.ascontiguousarray(np.asarray(inputs["b_comment"], np.float32))
    Ws = np.ascontiguousarray(np.asarray(inputs["W_srt"], np.float32))
    bs_ = np.ascontiguousarray(np.asarray(inputs["b_srt"], np.float32))

    wc1, wc2 = _f16_pair(Wc)
    ws1, ws2 = _f16_pair(Ws)
    hcT1, hcT2 = _f16_pair(hc.T)
    hsT1, hsT2 = _f16_pair(hs.T)

    OWN_BT = cfg.bs // P
    in_maps = []
    for c in range(cfg.n_cores):
        bsl = slice(c * cfg.bs, (c + 1) * cfg.bs)
        nsl = slice(c * cfg.ns, (c + 1) * cfg.ns)
        rmv = np.zeros((P, OWN_BT * cfg.n_cores), np.uint32)
        for obt in range(OWN_BT):
            for q in range(cfg.n_cores):
                rmv[:, obt * cfg.n_cores + q] = (
                    q * cfg.b + c * cfg.bs + obt * P + np.arange(P))
        in_maps.append({
            "wc1": wc1, "wc2": wc2,
            "hc1": np.ascontiguousarray(hcT1[:, bsl]),
            "hc2": np.ascontiguousarray(hcT2[:, bsl]),
            "ws1": ws1, "ws2": ws2,
            "hs1": np.ascontiguousarray(hsT1[:, nsl]),
            "hs2": np.ascontiguousarray(hsT2[:, nsl]),
            "ws_f32": Ws,
            "hs_full": hs,
            "bc": bc, "bs": bs_,
            "core_off": np.full((P, 1), c * cfg.ns, np.float32),
            "rm": rmv,
        })
    return in_maps


_BUILT = {}


def _get_nc(cfg: Cfg):
    if cfg not in _BUILT:
        _BUILT[cfg] = _build(cfg)
    return _BUILT[cfg]


def _run(inputs, cfg: Cfg = FULL, trace=False, trace_kwargs=None):
    nc = _get_nc(cfg)
    in_maps = _host_prep(inputs, cfg)
    res = run_bass_kernel_spmd(
        nc, in_maps, core_ids=list(range(cfg.n_cores)), trace=trace,
        **({"trace_kwargs": trace_kwargs} if trace_kwargs else {}))
    out = np.empty((cfg.b, 2, cfg.ds), np.float32)
    for c in range(cfg.n_cores):
        out[c * cfg.bs:(c + 1) * cfg.bs] = res.results[c]["sel"]
    return out, res


def kernel(**inputs) -> np.ndarray:
    k = int(inputs.get("k", 2))
    assert k == 2, f"kernel is specialized for k=2, got {k}"
    out, _ = _run(inputs, FULL, trace=False)
    return out


# revision 11
# speedup vs baseline: 1.2829x; 1.2829x over previous
"""Trainium2 Bass kernel for nn_Model3_4355096838495 (topk_masking).

Pipeline (8 NeuronCores, SPMD):
  relu-linear transforms -> dense scores -> top-2 srt rows per comment ->
  gather transformed rows.  Softmax is monotonic, so top-k on raw scores
  equals top-k on softmax weights.

Distribution:
- srt rows sharded (1024/core): exact transposed transform ts.T of the shard
  via 3-term fp16-pair matmuls (fp32-grade; PE rounds matmul operands to
  ~11 bits, so x1y1+x1y2+x2y1 of fp16 pairs is needed for exact ordering).
- comments sharded 512/core for the transform as b-tiles {c, 8+c, 16+c,
  24+c}; the fp16-pair tc.T shards are AllGathered (16.8MB) overlapped with
  the srt transform.
- scores [4096 x own-1024] via 3-term fp16-pair matmuls; ts.T is mean-centered
  (per-row constant shift of scores, ordering-invariant) so fp32 accumulation
  noise stays ~10x below knife-edge gaps; per-row top-8 via DVE max/max_index.
- per-quarter candidate AllGather + exact 64-candidate merge + value tail
  (indirect-gather hs rows, PE transpose, fp32r matmul + bias + relu),
  pipelined against the next quarter's scores.
"""
import os
import sys
from dataclasses import dataclass

import numpy as np

try:
    import concourse.bass as bass  # noqa: F401
except ImportError:  # pragma: no cover
    sys.path.insert(0, "/opt/trn_rl_repo")

import concourse.bass as bass
import concourse.mybir as mybir
from concourse import bacc
from concourse.tile import TileContext
from concourse.bass_utils import run_bass_kernel_spmd
from concourse.masks import make_identity

F32 = mybir.dt.float32
F32R = mybir.dt.float32r
F16 = mybir.dt.float16
U32 = mybir.dt.uint32
P = 128
# Sentinel for masked-index min-reduction: a power of two larger than any srt
# index, so (idx - BIG) + BIG is exact in fp32, and big enough that
# subtracting it from a score sinks the value below every real score.
BIG = 16384.0


@dataclass(frozen=True)
class Cfg:
    b: int = 4096
    n_srt: int = 8192
    dc: int = 768
    ds: int = 1024
    n_cores: int = 8

    @property
    def bs(self):
        return self.b // self.n_cores

    @property
    def ns(self):
        return self.n_srt // self.n_cores


FULL = Cfg()


def _build(cfg: Cfg):
    ncc = cfg.n_cores
    BS, NS = cfg.bs, cfg.ns

    nc = bacc.Bacc("TRN2", target_bir_lowering=False, debug=False,
                   num_devices=ncc)

    def din(name, shape, dt):
        return nc.dram_tensor(name, shape, dt, kind="ExternalInput").ap()

    v = {
        "wc1": din("wc1", [cfg.dc, cfg.ds], F16),
        "wc2": din("wc2", [cfg.dc, cfg.ds], F16),
        "hc1": din("hc1", [cfg.dc, BS], F16),
        "hc2": din("hc2", [cfg.dc, BS], F16),
        "ws1": din("ws1", [cfg.ds, cfg.ds], F16),
        "ws2": din("ws2", [cfg.ds, cfg.ds], F16),
        "hs1": din("hs1", [cfg.ds, NS], F16),
        "hs2": din("hs2", [cfg.ds, NS], F16),
        "ws_f32": din("ws_f32", [cfg.ds, cfg.ds], F32R),
        "hs_full": din("hs_full", [cfg.n_srt, cfg.ds], F32),
        "bc_in": din("bc", [cfg.ds], F32),
        "bs_in": din("bs", [cfg.ds], F32),
        "bs_r": din("bs_r", [cfg.ds], F32R),
        "ones_r": din("ones_r", [P], F32R),
        "nu_in": din("nu", [cfg.ds], F32),
        "core_off": din("core_off", [P, 1], F32),
        "rm": din("rm", [P, ncc], U32),
        "sel_out": nc.dram_tensor("sel", [BS, 2, cfg.ds], F32,
                                  kind="ExternalOutput").ap(),
    }

    with TileContext(nc) as tc:
        _body(tc, nc, cfg, v)
    nc.compile()
    return nc


def _body(tc, nc, cfg: Cfg, v):
    ncc = cfg.n_cores
    KT_C = cfg.dc // P
    KT_S = cfg.ds // P
    OT = cfg.ds // P
    BS, NS = cfg.bs, cfg.ns
    B_TILES = cfg.b // P
    OWN_BT = BS // P               # own b-tiles == quarters
    QW = B_TILES // OWN_BT         # b-tiles per quarter (== n_cores)
    N_CH = max(NS // 512, 1)
    NCW = min(512, NS)
    BCW = min(512, BS)
    B_CH = max(BS // 512, 1)
    OCW = min(512, cfg.ds)
    O_CH = max(cfg.ds // 512, 1)
    groups = [list(range(ncc))]

    from contextlib import ExitStack
    ctx = ExitStack()
    with ctx:
        const = ctx.enter_context(tc.tile_pool(name="const", bufs=1))
        dram = ctx.enter_context(tc.tile_pool(name="dram", bufs=1, space="DRAM"))

        ident = const.tile([P, P], F32)
        make_identity(nc, ident)
        ones_row = const.tile([1, P], F32R)
        nc.sync.dma_start(out=ones_row[:],
                          in_=v["ones_r"].rearrange("(o d) -> o d", o=1))
        b_com_sb = const.tile([P, OT], F32)
        nc.sync.dma_start(out=b_com_sb[:],
                          in_=v["bc_in"].rearrange("(t p) -> p t", p=P))
        b_srt_sb = const.tile([P, OT], F32)
        nc.sync.dma_start(out=b_srt_sb[:],
                          in_=v["bs_in"].rearrange("(t p) -> p t", p=P))
        nu_sb = const.tile([P, OT], F32)
        nc.sync.dma_start(out=nu_sb[:],
                          in_=v["nu_in"].rearrange("(t p) -> p t", p=P))
        b_srt_row = const.tile([1, cfg.ds], F32R)
        nc.sync.dma_start(out=b_srt_row[:],
                          in_=v["bs_r"].rearrange("(o d) -> o d", o=1))
        core_off_sb = const.tile([P, 1], F32)
        nc.sync.dma_start(out=core_off_sb[:], in_=v["core_off"][:])
        rm_sb = const.tile([P, ncc], U32)
        nc.sync.dma_start(out=rm_sb[:], in_=v["rm"][:])

        tc_b_in = dram.tile([2, cfg.ds, BS], F16)
        tc_ag = dram.tile([ncc, 2, cfg.ds, BS], F16, addr_space="Shared")
        cand_in = [dram.tile([QW * P, 16], F32, name=f"cand_in{q}")
                   for q in range(OWN_BT)]
        cand_ag = [dram.tile([ncc, QW * P, 16], F32, addr_space="Shared",
                             name=f"cand_ag{q}")
                   for q in range(OWN_BT)]

        # persistent tsT fp16 pair (centered)
        ts1_sb = const.tile([P, KT_S * NS], F16, tag="ts1")
        ts2_sb = const.tile([P, KT_S * NS], F16, tag="ts2")
        # fp32 W_srt for the tail (loaded during scores, used at quarter merges)
        ws32_sb = const.tile([P, KT_S * cfg.ds], F32R, tag="ws32")

        # ---- input pools (prefetched; closed when their phase ends) ----
        tcin = tc.tile_pool(name="tcin", bufs=1)
        tsin = tc.tile_pool(name="tsin", bufs=1)
        with tcin as tip, tsin as sip:
            wc1_sb = tip.tile([P, KT_C * cfg.ds], F16, tag="wc1")
            wc2_sb = tip.tile([P, KT_C * cfg.ds], F16, tag="wc2")
            hc1_sb = tip.tile([P, KT_C * BS], F16, tag="hc1")
            hc2_sb = tip.tile([P, KT_C * BS], F16, tag="hc2")
            ws1_sb = sip.tile([P, KT_S * cfg.ds], F16, tag="ws1")
            ws2_sb = sip.tile([P, KT_S * cfg.ds], F16, tag="ws2")
            hs1_sb = sip.tile([P, KT_S * NS], F16, tag="hs1")
            hs2_sb = sip.tile([P, KT_S * NS], F16, tag="hs2")
            for t_sb, t_dr, kt in ((wc1_sb, v["wc1"], KT_C), (wc2_sb, v["wc2"], KT_C),
                                   (hc1_sb, v["hc1"], KT_C), (hc2_sb, v["hc2"], KT_C),
                                   (ws1_sb, v["ws1"], KT_S), (ws2_sb, v["ws2"], KT_S),
                                   (hs1_sb, v["hs1"], KT_S), (hs2_sb, v["hs2"], KT_S)):
                nc.sync.dma_start(
                    out=t_sb[:].rearrange("p (k w) -> p k w", k=kt),
                    in_=t_dr.rearrange("(k p) w -> p k w", p=P))

            # ========== Phase 1: comment transform ==========
            with nc.named_scope("tc_phase"), \
                 tc.tile_pool(name="tcps", bufs=4, space="PSUM") as pp, \
                 tc.tile_pool(name="tcout", bufs=2) as op:
                for ot in range(OT):
                    for bc in range(B_CH):
                        ps = pp.tile([P, BCW], F32, tag="ps")
                        terms = ((wc1_sb, hc1_sb), (wc1_sb, hc2_sb), (wc2_sb, hc1_sb))
                        n_mm = len(terms) * KT_C
                        i = 0
                        for lt, rt in terms:
                            for k in range(KT_C):
                                nc.tensor.matmul(
                                    ps[:],
                                    lhsT=lt[:, k * cfg.ds + ot * P:k * cfg.ds + (ot + 1) * P],
                                    rhs=rt[:, k * BS + bc * BCW:k * BS + bc * BCW + BCW],
                                    start=(i == 0), stop=(i == n_mm - 1))
                                i += 1
                        tcx = op.tile([P, BCW], F32, tag="tcx")
                        nc.scalar.activation(tcx[:], ps[:],
                                             mybir.ActivationFunctionType.Relu,
                                             bias=b_com_sb[:, ot:ot + 1])
                        h1 = op.tile([P, BCW], F16, tag="h1")
                        nc.vector.tensor_copy(h1[:], tcx[:])
                        u1 = op.tile([P, BCW], F32, tag="u1")
                        nc.scalar.copy(u1[:], h1[:])
                        h2 = op.tile([P, BCW], F16, tag="h2")
                        nc.vector.tensor_tensor(out=h2[:], in0=tcx[:], in1=u1[:],
                                                op=mybir.AluOpType.subtract)
                        nc.sync.dma_start(
                            out=tc_b_in[0, ot * P:(ot + 1) * P, bc * BCW:bc * BCW + BCW],
                            in_=h1[:])
                        nc.sync.dma_start(
                            out=tc_b_in[1, ot * P:(ot + 1) * P, bc * BCW:bc * BCW + BCW],
                            in_=h2[:])

            with nc.named_scope("tc_allgather"):
                nc.gpsimd.collective_compute(
                    "AllGather", mybir.AluOpType.bypass, replica_groups=groups,
                    ins=[tc_b_in.opt()], outs=[tc_ag.opt()])

            # ========== Phase 2: srt transform -> centered tsT pair ==========
            with nc.named_scope("ts_phase"), \
                 tc.tile_pool(name="tsps", bufs=4, space="PSUM") as pp, \
                 tc.tile_pool(name="tsout", bufs=2) as op:
                for ot in range(OT):
                    for nch in range(N_CH):
                        ps = pp.tile([P, NCW], F32, tag="ps")
                        terms = ((ws1_sb, hs1_sb), (ws1_sb, hs2_sb), (ws2_sb, hs1_sb))
                        n_mm = len(terms) * KT_S
                        i = 0
                        for lt, rt in terms:
                            for k in range(KT_S):
                                nc.tensor.matmul(
                                    ps[:],
                                    lhsT=lt[:, k * cfg.ds + ot * P:k * cfg.ds + (ot + 1) * P],
                                    rhs=rt[:, k * NS + nch * NCW:k * NS + nch * NCW + NCW],
                                    start=(i == 0), stop=(i == n_mm - 1))
                                i += 1
                        tsx = op.tile([P, NCW], F32, tag="tsx")
                        nc.scalar.activation(tsx[:], ps[:],
                                             mybir.ActivationFunctionType.Relu,
                                             bias=b_srt_sb[:, ot:ot + 1])
                        # center: xm = relu(x+b) - nu  (ordering-invariant shift)
                        xm = op.tile([P, NCW], F32, tag="xm")
                        nc.vector.tensor_scalar(xm[:], tsx[:], nu_sb[:, ot:ot + 1],
                                                None, op0=mybir.AluOpType.subtract)
                        dst1 = ts1_sb[:, ot * NS + nch * NCW:ot * NS + nch * NCW + NCW]
                        nc.vector.tensor_copy(dst1, xm[:])
                        u1 = op.tile([P, NCW], F32, tag="u1")
                        nc.scalar.copy(u1[:], dst1)
                        dst2 = ts2_sb[:, ot * NS + nch * NCW:ot * NS + nch * NCW + NCW]
                        nc.vector.tensor_tensor(out=dst2, in0=xm[:], in1=u1[:],
                                                op=mybir.AluOpType.subtract)

            nc.sync.dma_start(
                out=ws32_sb[:].rearrange("p (k w) -> p k w", k=KT_S),
                in_=v["ws_f32"].rearrange("(k p) w -> p k w", p=P))

        # ========== Phase 3+4+5: scores / candidates / merge / tail,
        #            pipelined per quarter ==========
        with tc.tile_pool(name="sclhs", bufs=4) as lp, \
             tc.tile_pool(name="scps", bufs=4, space="PSUM") as pp, \
             tc.tile_pool(name="scsb", bufs=3) as sp, \
             tc.tile_pool(name="sccand", bufs=3) as cp, \
             tc.tile_pool(name="mg", bufs=2) as mp, \
             tc.tile_pool(name="mgsel", bufs=2) as selp, \
             tc.tile_pool(name="tlg", bufs=3) as gp, \
             tc.tile_pool(name="tlgt", bufs=2) as gtp, \
             tc.tile_pool(name="tlps", bufs=2, space="PSUM") as tpp, \
             tc.tile_pool(name="tlps2", bufs=2, space="PSUM") as tpp2, \
             tc.tile_pool(name="tlout", bufs=3) as top_:

            def do_scores_tile(bt):
                rank = bt % QW   # which core's tc_ag block holds this b-tile
                lb = bt // QW    # local tile index within that block (== quarter)
                lhs1 = lp.tile([P, KT_S * P], F16, tag="lhs1")
                lhs2 = lp.tile([P, KT_S * P], F16, tag="lhs2")
                nc.sync.dma_start(
                    out=lhs1[:].rearrange("p (k w) -> p k w", k=KT_S),
                    in_=tc_ag[rank, 0, :, lb * P:(lb + 1) * P]
                        .rearrange("(k p) w -> p k w", p=P))
                nc.sync.dma_start(
                    out=lhs2[:].rearrange("p (k w) -> p k w", k=KT_S),
                    in_=tc_ag[rank, 1, :, lb * P:(lb + 1) * P]
                        .rearrange("(k p) w -> p k w", p=P))
                scores = sp.tile([P, NS], F32, tag="scores")
                for nch in range(N_CH):
                    ps = pp.tile([P, NCW], F32, tag="ps")
                    terms = ((lhs1, ts1_sb), (lhs1, ts2_sb), (lhs2, ts1_sb))
                    n_mm = len(terms) * KT_S
                    i = 0
                    for lt, rt in terms:
                        for k in range(KT_S):
                            nc.tensor.matmul(
                                ps[:],
                                lhsT=lt[:, k * P:(k + 1) * P],
                                rhs=rt[:, k * NS + nch * NCW:k * NS + nch * NCW + NCW],
                                start=(i == 0), stop=(i == n_mm - 1))
                            i += 1
                    nc.scalar.copy(scores[:, nch * NCW:nch * NCW + NCW], ps[:])
                mx = cp.tile([P, 8], F32, tag="mx")
                nc.vector.max(out=mx[:], in_=scores[:])
                mi = cp.tile([P, 8], U32, tag="mi")
                nc.vector.max_index(out=mi[:], in_max=mx[:], in_values=scores[:])
                cand = cp.tile([P, 16], F32, tag="cand")
                nc.vector.tensor_copy(cand[:, 0:8], mx[:])
                nc.vector.tensor_copy(cand[:, 8:16], mi[:])
                nc.vector.tensor_scalar(
                    cand[:, 8:16], cand[:, 8:16], core_off_sb[:, 0:1], None,
                    op0=mybir.AluOpType.add)
                nc.sync.dma_start(out=cand_in[lb][rank * P:(rank + 1) * P, :],
                                  in_=cand[:])

            def do_merge_tail(q):
                cand_flat = cand_ag[q][:].rearrange("r b s -> (r b) s")
                ct = mp.tile([P, ncc * 16], F32, tag="ct")
                for r_ in range(ncc):
                    nc.gpsimd.indirect_dma_start(
                        out=ct[:, r_ * 16:(r_ + 1) * 16], out_offset=None,
                        in_=cand_flat,
                        in_offset=bass.IndirectOffsetOnAxis(
                            ap=rm_sb[:, r_:r_ + 1], axis=0))
                vals = ct[:].rearrange("p (r s) -> p r s", r=ncc)[:, :, 0:8]
                idxs = ct[:].rearrange("p (r s) -> p r s", r=ncc)[:, :, 8:16]
                nq = ncc * 8

                def bcast(x):
                    return x.rearrange("p (a o) -> p a o", o=1) \
                            .to_broadcast([P, ncc, 8])

                m1 = mp.tile([P, 1], F32, tag="m1")
                nc.vector.tensor_reduce(out=m1[:], in_=vals,
                                        axis=mybir.AxisListType.XY,
                                        op=mybir.AluOpType.max)
                eq = mp.tile([P, nq], F32, tag="eq")
                eqv = eq[:].rearrange("p (r s) -> p r s", r=ncc)
                nc.vector.tensor_tensor(out=eqv, in0=vals, in1=bcast(m1[:]),
                                        op=mybir.AluOpType.is_equal)
                t1 = mp.tile([P, nq], F32, tag="t1")
                t1v = t1[:].rearrange("p (r s) -> p r s", r=ncc)
                nc.vector.tensor_scalar(t1v, idxs, -BIG, None,
                                        op0=mybir.AluOpType.add)
                nc.vector.tensor_tensor(out=t1v, in0=t1v, in1=eqv,
                                        op=mybir.AluOpType.mult)
                nc.vector.tensor_scalar(t1v, t1v, BIG, None,
                                        op0=mybir.AluOpType.add)
                i1 = mp.tile([P, 1], F32, tag="i1")
                nc.vector.tensor_reduce(out=i1[:], in_=t1v,
                                        axis=mybir.AxisListType.XY,
                                        op=mybir.AluOpType.min)
                k1 = mp.tile([P, nq], F32, tag="k1")
                k1v = k1[:].rearrange("p (r s) -> p r s", r=ncc)
                nc.vector.tensor_tensor(out=k1v, in0=idxs, in1=bcast(i1[:]),
                                        op=mybir.AluOpType.is_equal)
                nc.vector.tensor_scalar(k1v, k1v, BIG, None,
                                        op0=mybir.AluOpType.mult)
                v2 = mp.tile([P, nq], F32, tag="v2")
                v2v = v2[:].rearrange("p (r s) -> p r s", r=ncc)
                nc.vector.tensor_tensor(out=v2v, in0=vals, in1=k1v,
                                        op=mybir.AluOpType.subtract)
                m2 = mp.tile([P, 1], F32, tag="m2")
                nc.vector.tensor_reduce(out=m2[:], in_=v2v,
                                        axis=mybir.AxisListType.XY,
                                        op=mybir.AluOpType.max)
                nc.vector.tensor_tensor(out=eqv, in0=v2v, in1=bcast(m2[:]),
                                        op=mybir.AluOpType.is_equal)
                nc.vector.tensor_scalar(t1v, idxs, -BIG, None,
                                        op0=mybir.AluOpType.add)
                nc.vector.tensor_tensor(out=t1v, in0=t1v, in1=eqv,
                                        op=mybir.AluOpType.mult)
                nc.vector.tensor_scalar(t1v, t1v, BIG, None,
                                        op0=mybir.AluOpType.add)
                i2 = mp.tile([P, 1], F32, tag="i2")
                nc.vector.tensor_reduce(out=i2[:], in_=t1v,
                                        axis=mybir.AxisListType.XY,
                                        op=mybir.AluOpType.min)
                sel_u32 = selp.tile([P, 2], U32, tag="sel")
                nc.vector.tensor_copy(sel_u32[:, 0:1], i1[:])
                nc.vector.tensor_copy(sel_u32[:, 1:2], i2[:])

                # tail: gather hs rows for both selections, transpose, matmul
                R = 2 * P
                hsgT = gtp.tile([P, KT_S * R], F32R, tag="hsgT")
                for j in range(2):
                    g = gp.tile([P, cfg.ds], F32, tag="g")
                    nc.gpsimd.indirect_dma_start(
                        out=g[:], out_offset=None,
                        in_=v["hs_full"][:],
                        in_offset=bass.IndirectOffsetOnAxis(
                            ap=sel_u32[:, j:j + 1], axis=0))
                    for dj in range(KT_S):
                        tp_ps = tpp2.tile([P, P], F32, tag="tp")
                        nc.tensor.transpose(out=tp_ps[:],
                                            in_=g[:, dj * P:(dj + 1) * P],
                                            identity=ident[:])
                        nc.vector.tensor_copy(
                            hsgT[:, dj * R + j * P:dj * R + (j + 1) * P],
                            tp_ps[:])
                for j in range(2):
                    for oc in range(O_CH):
                        ps = tpp.tile([P, OCW], F32, tag="mmps")
                        nc.tensor.matmul(
                            ps[:], lhsT=ones_row[0:1, :],
                            rhs=b_srt_row[0:1, oc * OCW:oc * OCW + OCW],
                            start=True, stop=False)
                        for k in range(KT_S):
                            nc.tensor.matmul(
                                ps[:],
                                lhsT=hsgT[:, k * R + j * P:k * R + (j + 1) * P],
                                rhs=ws32_sb[:, k * cfg.ds + oc * OCW:k * cfg.ds + oc * OCW + OCW],
                                start=False, stop=(k == KT_S - 1))
                        o_sb = top_.tile([P, OCW], F32, tag="osb")
                        nc.scalar.activation(o_sb[:], ps[:],
                                             mybir.ActivationFunctionType.Relu,
                                             bias=0.0)
                        nc.sync.dma_start(
                            out=v["sel_out"][q * P:(q + 1) * P, j,
                                             oc * OCW:oc * OCW + OCW],
                            in_=o_sb[:])

            for q in range(OWN_BT):
                with nc.named_scope(f"scores_q{q}"):
                    for bt in range(q * QW, (q + 1) * QW):
                        do_scores_tile(bt)
                with nc.named_scope(f"cand_ag_q{q}"):
                    nc.gpsimd.collective_compute(
                        "AllGather", mybir.AluOpType.bypass,
                        replica_groups=groups,
                        ins=[cand_in[q].opt()], outs=[cand_ag[q].opt()])
                with nc.named_scope(f"merge_tail_q{q}"):
                    do_merge_tail(q)


# ---------------------------------------------------------------------------
# host side
# ---------------------------------------------------------------------------

def _f16_pair(x):
    x = np.ascontiguousarray(x, np.float32)
    h1 = x.astype(np.float16)
    h2 = (x - h1.astype(np.float32)).astype(np.float16)
    return h1, h2


def _own_rows(cfg: Cfg, c):
    """Global comment rows owned by core c: b-tiles {c, QW+c, 2QW+c, ...}."""
    OWN_BT = cfg.bs // P
    QW = (cfg.b // P) // OWN_BT
    rows = []
    for q in range(OWN_BT):
        bt = q * QW + c
        rows.append(np.arange(bt * P, (bt + 1) * P))
    return np.concatenate(rows)


def _host_prep(inputs, cfg: Cfg):
    hs = np.ascontiguousarray(np.asarray(inputs["hidden_states_srt"], np.float32))
    hc = np.ascontiguousarray(np.asarray(inputs["hidden_states_comments"], np.float32))
    Wc = np.ascontiguousarray(np.asarray(inputs["W_comment"], np.float32))
    bc = np.ascontiguousarray(np.asarray(inputs["b_comment"], np.float32))
    Ws = np.ascontiguousarray(np.asarray(inputs["W_srt"], np.float32))
    bs_ = np.ascontiguousarray(np.asarray(inputs["b_srt"], np.float32))

    wc1, wc2 = _f16_pair(Wc)
    ws1, ws2 = _f16_pair(Ws)
    hcT = hc.T
    hsT1, hsT2 = _f16_pair(hs.T)

    # centering vector: approx column means of the transformed srt table from a
    # row subsample (any constant vector is ordering-correct; the mean just
    # minimizes accumulation noise).
    sub = hs[:: max(1, cfg.n_srt // 256)][:256]
    nu = np.maximum(sub @ Ws + bs_, 0).mean(axis=0).astype(np.float32)

    in_maps = []
    for c in range(cfg.n_cores):
        own = _own_rows(cfg, c)
        nsl = slice(c * cfg.ns, (c + 1) * cfg.ns)
        hcT1, hcT2 = _f16_pair(hcT[:, own])
        rmv = np.zeros((P, cfg.n_cores), np.uint32)
        for r in range(cfg.n_cores):
            rmv[:, r] = r * (cfg.b // (cfg.bs // P)) + c * P + np.arange(P)
        in_maps.append({
            "wc1": wc1, "wc2": wc2,
            "hc1": hcT1, "hc2": hcT2,
            "ws1": ws1, "ws2": ws2,
            "hs1": np.ascontiguousarray(hsT1[:, nsl]),
            "hs2": np.ascontiguousarray(hsT2[:, nsl]),
            "ws_f32": Ws,
            "hs_full": hs,
            "bc": bc, "bs": bs_, "bs_r": bs_, "nu": nu,
            "ones_r": np.ones(P, np.float32),
            "core_off": np.full((P, 1), c * cfg.ns, np.float32),
            "rm": rmv,
        })
    return in_maps


_BUILT = {}


def _get_nc(cfg: Cfg):
    if cfg not in _BUILT:
        _BUILT[cfg] = _build(cfg)
    return _BUILT[cfg]


def _run(inputs, cfg: Cfg = FULL, trace=False):
    nc = _get_nc(cfg)
    in_maps = _host_prep(inputs, cfg)
    res = run_bass_kernel_spmd(
        nc, in_maps, core_ids=list(range(cfg.n_cores)), trace=trace)
    out = np.empty((cfg.b, 2, cfg.ds), np.float32)
    for c in range(cfg.n_cores):
        out[_own_rows(cfg, c)] = res.results[c]["sel"]
    return out, res


def kernel(**inputs) -> np.ndarray:
    k = int(inputs.get("k", 2))
    assert k == 2, f"kernel is specialized for k=2, got {k}"
    out, _ = _run(inputs, FULL, trace=False)
    return out
